# revision 1
# baseline (speedup 1.0000x reference)
"""Causal self-attention (B=4, T=2048, C=768, H=12) on 8 trn2 NeuronCores.

Sharding: core c -> batch b = c//2, head-half hh = c%2 (6 heads per core).
Each core computes, for its (b, 6 heads): qkv projection, causal attention,
and a partial output projection (its heads' rows of W_proj). The host sums
the two partial outputs per batch and adds b_proj.

All matmuls run in float32r (tf32-like, ~13-bit mantissa, full PE rate at
N>=256). Layouts are chosen so the PE contracts over partitions everywhere
and softmax needs no transposes:
  - q^T, k^T [d, T]: weight-stationary qkv matmul
  - S^T [tk, tq] blocks: lhsT = k^T tile, rhs = q^T chunk; two heads of a
    pair run concurrently via row groups (base partitions 0/64, K=64)
  - exp on ACT straight out of PSUM ([128, 1024] pair tiles, causal-skipped)
  - PV: stationary [v_A | ones | v_B] per pair; the ones block makes each
    head's softmax denominator come out replicated on the 64 partitions
    opposite its O^T rows, so normalize = reciprocal + partition-swap DMA +
    elementwise multiply, no cross-partition vector ops.
"""

import numpy as np

B, T, C = 4, 2048, 768
H = 12
D = C // H          # 64
HPC = 6             # heads per core
NP = 3              # head pairs per core
N_CORES = 8
TK = T // 128       # 16 tk tiles
NCH = T // 512      # 4 tq chunks
CT = C // 128       # 6 contraction tiles

_cache = {}


def _build(has_bias):
    import concourse.tile as tile
    from concourse import bacc, mybir

    dt = mybir.dt
    r32 = dt.float32r
    f32 = dt.float32
    bf16 = dt.bfloat16
    Exp = mybir.ActivationFunctionType.Exp

    nc = bacc.Bacc("TRN2", target_bir_lowering=False, debug=False,
                   num_devices=N_CORES)

    xT_ap = nc.dram_tensor("xT", [C, T], r32, kind="ExternalInput").ap()
    wa_ap = nc.dram_tensor("wa", [C, 1152], r32, kind="ExternalInput").ap()
    wp_ap = nc.dram_tensor("wp", [HPC * D, C], r32, kind="ExternalInput").ap()
    tri_ap = nc.dram_tensor("tri", [128, 128], f32, kind="ExternalInput").ap()
    if has_bias:
        ox_ap = nc.dram_tensor("ox", [1, T], r32, kind="ExternalInput").ap()
        wb_ap = nc.dram_tensor("wb", [1, 1152], r32, kind="ExternalInput").ap()
    out_ap = nc.dram_tensor("out", [T, C], f32, kind="ExternalOutput").ap()
    import os as _os
    dbg = bool(_os.environ.get("KV_DEBUG"))
    if dbg:
        dq_ap = nc.dram_tensor("dq", [128, T], r32, kind="ExternalOutput").ap()
        dk_ap = nc.dram_tensor("dk", [128, T], r32, kind="ExternalOutput").ap()
        pass
        dy_ap = nc.dram_tensor("dy", [128, T], r32, kind="ExternalOutput").ap()
        dpa_ap = nc.dram_tensor("dpa", [128, 512], f32, kind="ExternalOutput").ap()
        dpb_ap = nc.dram_tensor("dpb", [128, 512], f32, kind="ExternalOutput").ap()
        ds1_ap = nc.dram_tensor("ds1", [128, 512], f32, kind="ExternalOutput").ap()
        ds2_ap = nc.dram_tensor("ds2", [128, 512], f32, kind="ExternalOutput").ap()
        dP_ap = nc.dram_tensor("dP", [128, 1024], r32, kind="ExternalOutput").ap()

    with tile.TileContext(nc) as tc:
        with tc.tile_pool(name="pers", bufs=1) as pers, \
             tc.tile_pool(name="pP", bufs=3) as pP, \
             tc.tile_pool(name="pst", bufs=2) as pst, \
             tc.tile_pool(name="pout", bufs=2) as pout, \
             tc.tile_pool(name="psA", bufs=2, space="PSUM") as psA, \
             tc.tile_pool(name="psB", bufs=4, space="PSUM") as psB:

            # ---- persistent SBUF tensors + input DMA ----
            xT = [pers.tile([128, T], r32, tag=f"x{i}", name=f"x{i}") for i in range(CT)]
            wa = [pers.tile([128, 1152], r32, tag=f"w{i}", name=f"w{i}") for i in range(CT)]
            for i in range(CT):
                nc.sync.dma_start(xT[i][:], xT_ap[i * 128:(i + 1) * 128, :])
                nc.sync.dma_start(wa[i][:], wa_ap[i * 128:(i + 1) * 128, :])
            wp = [pers.tile([128, C], r32, tag=f"p{i}", name=f"wp{i}") for i in range(NP)]
            for i in range(NP):
                nc.sync.dma_start(wp[i][:], wp_ap[i * 128:(i + 1) * 128, :])
            tri = pers.tile([128, 128], f32, tag="tri")
            nc.sync.dma_start(tri[:], tri_ap)
            if has_bias:
                ox = pers.tile([1, T], r32, tag="ox")
                nc.sync.dma_start(ox[:], ox_ap)
                wb = pers.tile([1, 1152], r32, tag="wb")
                nc.sync.dma_start(wb[:], wb_ap)
            ones_f = pers.tile([128, 64], f32, tag="onesf")
            nc.vector.memset(ones_f[:], 1.0)
            ones_b = pers.tile([128, 64], bf16, tag="onesb")
            nc.vector.tensor_copy(ones_b[:], ones_f[:])
            tri_b = pers.tile([128, 128], bf16, tag="trib")
            nc.vector.tensor_copy(tri_b[:], tri[:])

            qk = [pers.tile([128, T], r32, tag=f"qk{i}", name=f"qk{i}") for i in range(2 * NP)]
            # v: one contiguous [128, 64] bf16 tile per (t-tile, head) — bf16
            # stationaries must be dense tiles (sliced wide tiles hit a slow
            # non-FWL weight-load path).
            vv = [[pers.tile([128, 64], bf16, tag=f"v{t}_{h}", name=f"v{t}_{h}")
                   for h in range(HPC)] for t in range(TK)]
            # Y^T for pair p: fresh tile for p=0; pairs 1 and 2 reuse the
            # q^T tiles of pairs 0 and 1, which are dead by the time attn
            # p starts writing (Tile tracks the WAR dependency).
            y0 = pers.tile([128, T], r32, tag="y0", name="y0")
            yt = [y0, qk[0], qk[2]]

            # ---- phase 1: qkv projections ----
            # q^T / k^T: stationary = wa column block, moving = xT chunk.
            # out tile order: pair0 q, pair0 k, pair1 q, ... so attention on
            # pair p can start as early as possible.
            with nc.named_scope("qkv_qk"):
                for p in range(NP):
                    for qsel in range(2):          # 0 = q, 1 = k
                        dst = qk[2 * p + qsel]
                        wcol = qsel * 384 + p * 128
                        for cp in range(2):        # chunk pairs (1024 cols)
                            ps = psA.tile([128, 1024], f32, tag="A")
                            for half in range(2):
                                t0 = cp * 1024 + half * 512
                                for c in range(CT):
                                    nc.tensor.matmul(
                                        ps[:, half * 512:half * 512 + 512],
                                        lhsT=wa[c][:, wcol:wcol + 128],
                                        rhs=xT[c][:, t0:t0 + 512],
                                        start=(c == 0),
                                        stop=(c == CT - 1 and not has_bias))
                                if has_bias:
                                    nc.tensor.matmul(
                                        ps[:, half * 512:half * 512 + 512],
                                        lhsT=wb[0:1, wcol:wcol + 128],
                                        rhs=ox[0:1, t0:t0 + 512],
                                        start=False, stop=True)
                            nc.vector.tensor_copy(
                                dst[:, cp * 1024:(cp + 1) * 1024], ps[:])

            # v: stationary = xT[c] t-tile, moving = wa v columns.
            with nc.named_scope("qkv_v"):
                for t in range(TK):
                    ps = psB.tile([128, 512], f32, tag="B")
                    for c in range(CT):
                        nc.tensor.matmul(
                            ps[:, 0:384],
                            lhsT=xT[c][:, t * 128:(t + 1) * 128],
                            rhs=wa[c][:, 768:1152],
                            start=(c == 0),
                            stop=(c == CT - 1 and not has_bias))
                    if has_bias:
                        nc.tensor.matmul(
                            ps[:, 0:384],
                            lhsT=ox[0:1, t * 128:(t + 1) * 128],
                            rhs=wb[0:1, 768:1152],
                            start=False, stop=True)
                    for h in range(HPC):
                        nc.vector.tensor_copy(
                            vv[t][h][:], ps[:, h * 64:(h + 1) * 64])

            if dbg:
                nc.sync.dma_start(dq_ap, qk[0][:])
                nc.sync.dma_start(dk_ap, qk[1][:])

            # ---- phase 2: attention per head pair ----
            for p in range(NP):
                qA = qk[2 * p]
                kA = qk[2 * p + 1]
                with nc.named_scope(f"attn{p}"):
                    for j in range(NCH):
                        nblk = 4 * j + 4
                        pvA = psB.tile([128, 512], f32, tag="B")
                        pvB = psB.tile([128, 512], f32, tag="B")
                        Ps = [None] * nblk
                        ms = [None] * nblk

                        def emit_S(i):
                            m = i - 4 * j
                            lo = 128 * m if m >= 0 else 0
                            w = 512 - lo
                            sp = psA.tile([128, 1024], f32, tag="A")
                            for ab in range(2):
                                nc.tensor.matmul(
                                    sp[:, ab * 512 + lo:(ab + 1) * 512],
                                    lhsT=kA[ab * 64:(ab + 1) * 64,
                                            i * 128:(i + 1) * 128],
                                    rhs=qA[ab * 64:(ab + 1) * 64,
                                           j * 512 + lo:(j + 1) * 512],
                                    start=True, stop=True)
                            P = pP.tile([128, 1024], bf16, tag="P")
                            if lo:
                                nc.scalar.activation(
                                    P[:, lo:512], sp[:, lo:512], Exp)
                                nc.scalar.activation(
                                    P[:, 512 + lo:1024], sp[:, 512 + lo:1024],
                                    Exp)
                            else:
                                nc.scalar.activation(P[:], sp[:], Exp)
                            Ps[i], ms[i] = P, max(m, 0)

                        def emit_PV(i):
                            m = ms[i]
                            lo = 128 * m
                            P = Ps[i]
                            if m > 0 or i == 4 * j:
                                # diagonal sub-block masking (multiply by tri)
                                for ab in range(2):
                                    sl = P[:, ab * 512 + lo:ab * 512 + lo + 128]
                                    nc.vector.tensor_mul(sl, sl, tri_b[:])
                            first, last = (i == 0), (i == nblk - 1)
                            # head A: O on partitions 0-63, denom (ones) on
                            # 64-127; head B mirrored. O and denom matmuls
                            # land in different column groups and overlap.
                            nc.tensor.matmul(
                                pvA[0:64, lo:512], lhsT=vv[i][2 * p][:],
                                rhs=P[:, lo:512], start=first, stop=last)
                            nc.tensor.matmul(
                                pvA[64:128, lo:512], lhsT=ones_b[:],
                                rhs=P[:, lo:512], start=first, stop=last)
                            nc.tensor.matmul(
                                pvB[0:64, lo:512], lhsT=ones_b[:],
                                rhs=P[:, 512 + lo:1024], start=first, stop=last)
                            nc.tensor.matmul(
                                pvB[64:128, lo:512], lhsT=vv[i][2 * p + 1][:],
                                rhs=P[:, 512 + lo:1024], start=first, stop=last)

                        # software-pipeline: S(i+1) is emitted before PV(i)
                        emit_S(0)
                        for i in range(1, nblk):
                            emit_S(i)
                            emit_PV(i - 1)
                        emit_PV(nblk - 1)

                        # normalize. reciprocal_approx_fast only works at
                        # base partition 0, so: head B's denom (rows 0-63)
                        # is recip'd in place; head A's denom (rows 64-127)
                        # is staged, swapped down via DMA, then recip'd.
                        s1 = pst.tile([128, 512], f32, tag="st")
                        nc.vector.tensor_copy(s1[64:128, :], pvA[64:128, :])
                        nc.vector.reciprocal_approx_fast(
                            s1[0:64, :], pvB[0:64, :])
                        s2 = pst.tile([128, 512], f32, tag="rc")
                        nc.sync.dma_start(s2[0:64, :], s1[64:128, :])
                        nc.sync.dma_start(s2[64:128, :], s1[0:64, :])
                        s3 = pst.tile([64, 512], f32, tag="s3")
                        nc.vector.reciprocal_approx_fast(
                            s3[:], s2[0:64, :])
                        if dbg and p == 0 and j == 0:
                            dpa = pout.tile([128, 512], f32, tag="o", name="dpa")
                            nc.vector.tensor_copy(dpa[:], pvA[:])
                            nc.sync.dma_start(dpa_ap, dpa[:])
                            dpb = pout.tile([128, 512], f32, tag="o", name="dpb")
                            nc.vector.tensor_copy(dpb[:], pvB[:])
                            nc.sync.dma_start(dpb_ap, dpb[:])
                            nc.sync.dma_start(ds1_ap, s1[:])
                            nc.sync.dma_start(ds2_ap, s2[:])
                            nc.sync.dma_start(dP_ap, Ps[0][:])
                        nc.vector.tensor_mul(
                            yt[p][0:64, j * 512:(j + 1) * 512],
                            pvA[0:64, :], s3[:])
                        nc.vector.tensor_mul(
                            yt[p][64:128, j * 512:(j + 1) * 512],
                            pvB[64:128, :], s2[64:128, :])

            if dbg:
                nc.sync.dma_start(dy_ap, yt[0][:])

            # ---- phase 3: output projection (partial; host adds b_proj) ----
            with nc.named_scope("proj"):
                for t in range(TK):
                    ps = psA.tile([128, 1024], f32, tag="A")
                    for n0, n1 in ((0, 512), (512, 768)):
                        for kk in range(NP):
                            nc.tensor.matmul(
                                ps[:, n0:n1],
                                lhsT=yt[kk][:, t * 128:(t + 1) * 128],
                                rhs=wp[kk][:, n0:n1],
                                start=(kk == 0), stop=(kk == NP - 1))
                    ob = pout.tile([128, C], f32, tag="o")
                    nc.vector.tensor_copy(ob[:], ps[:, 0:C])
                    nc.sync.dma_start(
                        out_ap[t * 128:(t + 1) * 128, :], ob[:])

    nc.compile()
    return nc


def _prep_inputs(x, W_qkv, b_qkv, W_proj):
    """Per-core input maps (numpy, float32 bits; fp32r tensors reuse them)."""
    sc = 1.0 / np.sqrt(D)
    tri = np.triu(np.ones((128, 128), dtype=np.float32))
    in_maps = []
    for c in range(N_CORES):
        b, hh = c // 2, c % 2
        h0 = hh * 384                      # column offset of this half's heads
        wq = W_qkv[:, h0:h0 + 384] * sc
        wk = W_qkv[:, 768 + h0:768 + h0 + 384]
        wv = W_qkv[:, 1536 + h0:1536 + h0 + 384]
        wa = np.ascontiguousarray(
            np.concatenate([wq, wk, wv], axis=1), dtype=np.float32)
        m = {
            "xT": np.ascontiguousarray(x[b].T, dtype=np.float32),
            "wa": wa,
            "wp": np.ascontiguousarray(W_proj[h0:h0 + 384, :], np.float32),
            "tri": tri,
        }
        if np.any(b_qkv):
            bq = b_qkv[h0:h0 + 384] * sc
            bk = b_qkv[768 + h0:768 + h0 + 384]
            bv = b_qkv[1536 + h0:1536 + h0 + 384]
            m["ox"] = np.ones((1, T), dtype=np.float32)
            m["wb"] = np.concatenate([bq, bk, bv]).reshape(1, 1152).astype(
                np.float32)
        in_maps.append(m)
    return in_maps


def _run(inputs, trace=False, tmpdir=None):
    from concourse.bass_utils import run_bass_kernel_spmd

    x = np.asarray(inputs["x"], dtype=np.float32)
    W_qkv = np.asarray(inputs["W_qkv"], dtype=np.float32)
    b_qkv = np.asarray(inputs["b_qkv"], dtype=np.float32)
    W_proj = np.asarray(inputs["W_proj"], dtype=np.float32)
    b_proj = np.asarray(inputs["b_proj"], dtype=np.float32)

    has_bias = bool(np.any(b_qkv))
    key = ("k", has_bias)
    if key not in _cache:
        _cache[key] = _build(has_bias)
    nc = _cache[key]

    in_maps = _prep_inputs(x, W_qkv, b_qkv, W_proj)
    res = run_bass_kernel_spmd(nc, in_maps, list(range(N_CORES)),
                               trace=trace, tmpdir=tmpdir)
    out = np.empty((B, T, C), dtype=np.float32)
    for b in range(B):
        out[b] = res.results[2 * b]["out"] + res.results[2 * b + 1]["out"]
    out += b_proj
    return out, res


def kernel(**inputs):
    out, _ = _run(inputs)
    return out



# revision 2
# speedup vs baseline: 1.0226x; 1.0226x over previous
"""Causal self-attention (B=4, T=2048, C=768, H=12) on 8 trn2 NeuronCores.

Sharding: core c -> batch b = c//2, head-half hh = c%2 (6 heads per core).
Each core computes, for its (b, 6 heads): qkv projection, causal attention,
and a partial output projection (its heads' rows of W_proj). The host sums
the two partial outputs per batch and adds b_proj.

v1 changes over the 321us baseline:
  - xT / wa inputs in float16 (half the input DMA; PE runs fp16 at full
    rate; 10-bit mantissa keeps quantization at ~5e-4).
  - v / P / tri / ones tiles in fp16 instead of bf16 (same speed, more
    mantissa).
  - proj is inlined per chunk inside pair-2's attention loop, with the
    output DMA per t-tile, so the old 45us proj+DMA tail overlaps
    attention.
  - diagonal-block masking (tri multiply) moved from Vector to the idle
    GpSimd engine.
  - partial-block exp merged into one ACTIVATE spanning [lo:1024] when
    that's cheaper than two calls (the garbage middle is masked later
    anyway).

Layouts (unchanged): q^T/k^T [d, T] fp32r; S^T [tk, tq] via row-group
pairs; PV with [v | ones] stationaries so each head's softmax denominator
lands on the partitions opposite its O^T rows (normalize = reciprocal +
partition-swap DMA + multiply).
"""

import numpy as np

B, T, C = 4, 2048, 768
H = 12
D = C // H          # 64
HPC = 6             # heads per core
NP = 3              # head pairs per core
N_CORES = 8
TK = T // 128       # 16 tk tiles
NCH = T // 512      # 4 tq chunks
CT = C // 128       # 6 contraction tiles

_cache = {}


def _build(has_bias):
    import concourse.tile as tile
    from concourse import bacc, mybir

    dt = mybir.dt
    r32 = dt.float32r
    f32 = dt.float32
    f16 = dt.float16
    Exp = mybir.ActivationFunctionType.Exp

    nc = bacc.Bacc("TRN2", target_bir_lowering=False, debug=False,
                   num_devices=N_CORES)

    xT_ap = nc.dram_tensor("xT", [C, T], f16, kind="ExternalInput").ap()
    wa_ap = nc.dram_tensor("wa", [C, 1152], f16, kind="ExternalInput").ap()
    wp_ap = nc.dram_tensor("wp", [HPC * D, C], r32, kind="ExternalInput").ap()
    tri_ap = nc.dram_tensor("tri", [128, 128], f32, kind="ExternalInput").ap()
    if has_bias:
        ox_ap = nc.dram_tensor("ox", [1, T], f16, kind="ExternalInput").ap()
        wb_ap = nc.dram_tensor("wb", [1, 1152], f16, kind="ExternalInput").ap()
    out_ap = nc.dram_tensor("out", [T, C], f32, kind="ExternalOutput").ap()

    with tile.TileContext(nc) as tc:
        with tc.tile_pool(name="pers", bufs=1) as pers, \
             tc.tile_pool(name="pP", bufs=3) as pP, \
             tc.tile_pool(name="pst", bufs=2) as pst, \
             tc.tile_pool(name="pout", bufs=2) as pout, \
             tc.tile_pool(name="psA", bufs=2, space="PSUM") as psA, \
             tc.tile_pool(name="psB", bufs=4, space="PSUM") as psB:

            # ---- persistent SBUF tensors + input DMA ----
            xT = [pers.tile([128, T], f16, tag=f"x{i}", name=f"x{i}") for i in range(CT)]
            wa = [pers.tile([128, 1152], f16, tag=f"w{i}", name=f"w{i}") for i in range(CT)]
            for i in range(CT):
                nc.sync.dma_start(xT[i][:], xT_ap[i * 128:(i + 1) * 128, :])
                nc.sync.dma_start(wa[i][:], wa_ap[i * 128:(i + 1) * 128, :])
            wp = [pers.tile([128, C], r32, tag=f"p{i}", name=f"wp{i}") for i in range(NP)]
            for i in range(NP):
                nc.sync.dma_start(wp[i][:], wp_ap[i * 128:(i + 1) * 128, :])
            tri = pers.tile([128, 128], f32, tag="tri")
            nc.sync.dma_start(tri[:], tri_ap)
            if has_bias:
                ox = pers.tile([1, T], f16, tag="ox")
                nc.sync.dma_start(ox[:], ox_ap)
                wb = pers.tile([1, 1152], f16, tag="wb")
                nc.sync.dma_start(wb[:], wb_ap)
            ones_f = pers.tile([128, 64], f32, tag="onesf")
            nc.vector.memset(ones_f[:], 1.0)
            ones_h = pers.tile([128, 64], f16, tag="onesh")
            nc.vector.tensor_copy(ones_h[:], ones_f[:])
            tri_h = pers.tile([128, 128], f16, tag="trih")
            nc.vector.tensor_copy(tri_h[:], tri[:])

            qk = [pers.tile([128, T], r32, tag=f"qk{i}", name=f"qk{i}") for i in range(2 * NP)]
            # v: one contiguous [128, 64] f16 tile per (t-tile, head) — 16-bit
            # stationaries must be dense tiles (sliced wide tiles hit a slow
            # non-FWL weight-load path).
            vv = [[pers.tile([128, 64], f16, tag=f"v{t}_{h}", name=f"v{t}_{h}")
                   for h in range(HPC)] for t in range(TK)]
            # Y^T for pair p: fresh tile for p=0; pairs 1 and 2 reuse the
            # q^T tiles of pairs 0 and 1, which are dead by the time attn
            # p starts writing (Tile tracks the WAR dependency).
            y0 = pers.tile([128, T], r32, tag="y0", name="y0")
            yt = [y0, qk[0], qk[2]]

            # ---- phase 1: qkv projections ----
            # q^T / k^T: stationary = wa column block, moving = xT chunk.
            # out tile order: pair0 q, pair0 k, pair1 q, ... so attention on
            # pair p can start as early as possible.
            with nc.named_scope("qkv_qk"):
                for p in range(NP):
                    for qsel in range(2):          # 0 = q, 1 = k
                        dst = qk[2 * p + qsel]
                        wcol = qsel * 384 + p * 128
                        for cp in range(2):        # chunk pairs (1024 cols)
                            ps = psA.tile([128, 1024], f32, tag="A")
                            for half in range(2):
                                t0 = cp * 1024 + half * 512
                                for c in range(CT):
                                    nc.tensor.matmul(
                                        ps[:, half * 512:half * 512 + 512],
                                        lhsT=wa[c][:, wcol:wcol + 128],
                                        rhs=xT[c][:, t0:t0 + 512],
                                        start=(c == 0),
                                        stop=(c == CT - 1 and not has_bias))
                                if has_bias:
                                    nc.tensor.matmul(
                                        ps[:, half * 512:half * 512 + 512],
                                        lhsT=wb[0:1, wcol:wcol + 128],
                                        rhs=ox[0:1, t0:t0 + 512],
                                        start=False, stop=True)
                            nc.vector.tensor_copy(
                                dst[:, cp * 1024:(cp + 1) * 1024], ps[:])

            # v: stationary = xT[c] t-tile, moving = wa v columns.
            with nc.named_scope("qkv_v"):
                for t in range(TK):
                    ps = psB.tile([128, 512], f32, tag="B")
                    for c in range(CT):
                        nc.tensor.matmul(
                            ps[:, 0:384],
                            lhsT=xT[c][:, t * 128:(t + 1) * 128],
                            rhs=wa[c][:, 768:1152],
                            start=(c == 0),
                            stop=(c == CT - 1 and not has_bias))
                    if has_bias:
                        nc.tensor.matmul(
                            ps[:, 0:384],
                            lhsT=ox[0:1, t * 128:(t + 1) * 128],
                            rhs=wb[0:1, 768:1152],
                            start=False, stop=True)
                    for h in range(HPC):
                        nc.vector.tensor_copy(
                            vv[t][h][:], ps[:, h * 64:(h + 1) * 64])

            # ---- phase 2: attention per head pair ----
            # (pair 2's loop also carries the inlined output projection)
            def emit_proj(j):
                # proj for the 4 t-tiles covered by q-chunk j; needs yt of
                # all three pairs at those columns, which pair-2 chunk j's
                # normalize has just written.
                for t in range(4 * j, 4 * j + 4):
                    ob = pout.tile([128, C], f32, tag="o")
                    for half, (n0, n1) in enumerate(((0, 512), (512, 768))):
                        pp = psB.tile([128, 512], f32, tag="B")
                        for kk in range(NP):
                            nc.tensor.matmul(
                                pp[:, 0:n1 - n0],
                                lhsT=yt[kk][:, t * 128:(t + 1) * 128],
                                rhs=wp[kk][:, n0:n1],
                                start=(kk == 0), stop=(kk == NP - 1))
                        nc.vector.tensor_copy(ob[:, n0:n1], pp[:, 0:n1 - n0])
                    nc.sync.dma_start(
                        out_ap[t * 128:(t + 1) * 128, :], ob[:])

            for p in range(NP):
                qA = qk[2 * p]
                kA = qk[2 * p + 1]
                with nc.named_scope(f"attn{p}"):
                    for j in range(NCH):
                        nblk = 4 * j + 4
                        pvA = psB.tile([128, 512], f32, tag="B")
                        pvB = psB.tile([128, 512], f32, tag="B")
                        Ps = [None] * nblk
                        ms = [None] * nblk

                        def emit_S(i):
                            m = i - 4 * j
                            lo = 128 * m if m >= 0 else 0
                            sp = psA.tile([128, 1024], f32, tag="A")
                            for ab in range(2):
                                nc.tensor.matmul(
                                    sp[:, ab * 512 + lo:(ab + 1) * 512],
                                    lhsT=kA[ab * 64:(ab + 1) * 64,
                                            i * 128:(i + 1) * 128],
                                    rhs=qA[ab * 64:(ab + 1) * 64,
                                           j * 512 + lo:(j + 1) * 512],
                                    start=True, stop=True)
                            P = pP.tile([128, 1024], f16, tag="P")
                            if lo == 0:
                                nc.scalar.activation(P[:], sp[:], Exp)
                            elif lo <= 256:
                                # one call across the (masked-later) middle
                                nc.scalar.activation(
                                    P[:, lo:1024], sp[:, lo:1024], Exp)
                            else:
                                nc.scalar.activation(
                                    P[:, lo:512], sp[:, lo:512], Exp)
                                nc.scalar.activation(
                                    P[:, 512 + lo:1024], sp[:, 512 + lo:1024],
                                    Exp)
                            Ps[i], ms[i] = P, max(m, 0)

                        def emit_PV(i):
                            m = ms[i]
                            lo = 128 * m
                            P = Ps[i]
                            if m > 0 or i == 4 * j:
                                # diagonal sub-block masking (multiply by tri)
                                for ab in range(2):
                                    sl = P[:, ab * 512 + lo:ab * 512 + lo + 128]
                                    nc.gpsimd.tensor_mul(sl, sl, tri_h[:])
                            first, last = (i == 0), (i == nblk - 1)
                            # head A: O on partitions 0-63, denom (ones) on
                            # 64-127; head B mirrored. O and denom matmuls
                            # land in different column groups and overlap.
                            nc.tensor.matmul(
                                pvA[0:64, lo:512], lhsT=vv[i][2 * p][:],
                                rhs=P[:, lo:512], start=first, stop=last)
                            nc.tensor.matmul(
                                pvA[64:128, lo:512], lhsT=ones_h[:],
                                rhs=P[:, lo:512], start=first, stop=last)
                            nc.tensor.matmul(
                                pvB[0:64, lo:512], lhsT=ones_h[:],
                                rhs=P[:, 512 + lo:1024], start=first, stop=last)
                            nc.tensor.matmul(
                                pvB[64:128, lo:512], lhsT=vv[i][2 * p + 1][:],
                                rhs=P[:, 512 + lo:1024], start=first, stop=last)

                        # software-pipeline: S(i+1) is emitted before PV(i)
                        emit_S(0)
                        for i in range(1, nblk):
                            emit_S(i)
                            emit_PV(i - 1)
                        emit_PV(nblk - 1)

                        # normalize. reciprocal_approx_fast only works at
                        # base partition 0, so: head B's denom (rows 0-63)
                        # is recip'd in place; head A's denom (rows 64-127)
                        # is staged, swapped down via DMA, then recip'd.
                        s1 = pst.tile([128, 512], f32, tag="st")
                        nc.vector.tensor_copy(s1[64:128, :], pvA[64:128, :])
                        nc.vector.reciprocal_approx_fast(
                            s1[0:64, :], pvB[0:64, :])
                        s2 = pst.tile([128, 512], f32, tag="rc")
                        nc.sync.dma_start(s2[0:64, :], s1[64:128, :])
                        nc.sync.dma_start(s2[64:128, :], s1[0:64, :])
                        s3 = pst.tile([64, 512], f32, tag="s3")
                        nc.vector.reciprocal_approx_fast(
                            s3[:], s2[0:64, :])
                        nc.vector.tensor_mul(
                            yt[p][0:64, j * 512:(j + 1) * 512],
                            pvA[0:64, :], s3[:])
                        nc.vector.tensor_mul(
                            yt[p][64:128, j * 512:(j + 1) * 512],
                            pvB[64:128, :], s2[64:128, :])

                        if p == NP - 1:
                            with nc.named_scope("proj"):
                                emit_proj(j)

    nc.compile()
    return nc


def _prep_inputs(x, W_qkv, b_qkv, W_proj):
    """Per-core input maps (numpy; xT/wa as float16)."""
    sc = 1.0 / np.sqrt(D)
    tri = np.triu(np.ones((128, 128), dtype=np.float32))
    in_maps = []
    for c in range(N_CORES):
        b, hh = c // 2, c % 2
        h0 = hh * 384                      # column offset of this half's heads
        wq = W_qkv[:, h0:h0 + 384] * sc
        wk = W_qkv[:, 768 + h0:768 + h0 + 384]
        wv = W_qkv[:, 1536 + h0:1536 + h0 + 384]
        wa = np.ascontiguousarray(
            np.concatenate([wq, wk, wv], axis=1), dtype=np.float16)
        m = {
            "xT": np.ascontiguousarray(x[b].T, dtype=np.float16),
            "wa": wa,
            "wp": np.ascontiguousarray(W_proj[h0:h0 + 384, :], np.float32),
            "tri": tri,
        }
        if np.any(b_qkv):
            bq = b_qkv[h0:h0 + 384] * sc
            bk = b_qkv[768 + h0:768 + h0 + 384]
            bv = b_qkv[1536 + h0:1536 + h0 + 384]
            m["ox"] = np.ones((1, T), dtype=np.float16)
            m["wb"] = np.concatenate([bq, bk, bv]).reshape(1, 1152).astype(
                np.float16)
        in_maps.append(m)
    return in_maps


def _run(inputs, trace=False, tmpdir=None):
    from concourse.bass_utils import run_bass_kernel_spmd

    x = np.asarray(inputs["x"], dtype=np.float32)
    W_qkv = np.asarray(inputs["W_qkv"], dtype=np.float32)
    b_qkv = np.asarray(inputs["b_qkv"], dtype=np.float32)
    W_proj = np.asarray(inputs["W_proj"], dtype=np.float32)
    b_proj = np.asarray(inputs["b_proj"], dtype=np.float32)

    has_bias = bool(np.any(b_qkv))
    key = ("k", has_bias)
    if key not in _cache:
        _cache[key] = _build(has_bias)
    nc = _cache[key]

    in_maps = _prep_inputs(x, W_qkv, b_qkv, W_proj)
    res = run_bass_kernel_spmd(nc, in_maps, list(range(N_CORES)),
                               trace=trace, tmpdir=tmpdir)
    out = np.empty((B, T, C), dtype=np.float32)
    for b in range(B):
        out[b] = res.results[2 * b]["out"] + res.results[2 * b + 1]["out"]
    out += b_proj
    return out, res


def kernel(**inputs):
    out, _ = _run(inputs)
    return out


# revision 3
# speedup vs baseline: 1.0648x; 1.0413x over previous
"""Causal self-attention (B=4, T=2048, C=768, H=12) on 8 trn2 NeuronCores.

Sharding: core c -> batch b = c//2, head-half hh = c%2 (6 heads per core).
Each core computes, for its (b, 6 heads): qkv projection, causal attention,
and a partial output projection (its heads' rows of W_proj). The host sums
the two partial outputs per batch and adds b_proj.

v2 over the 321us baseline:
  - xT / wa inputs in float16 (half the input DMA; fp16 matmuls run at
    full PE rate and keep quantization at ~5e-4).
  - causal masking of diagonal 128-blocks happens INSIDE the S matmul
    accumulation group: an extra N=128 matmul adds -30 to the
    above-diagonal entries (lhsT = -30*I, rhs = strict-lower pattern),
    so exp() maps them to 0 and no post-exp mask op exists at all.
  - softmax exp is split across two engines: ACT (scalar) exp for
    diagonal blocks + ~2/3 of clean blocks, and a Schraudolph
    bits-of-fp16 exp (one Vector tensor_scalar: bits = S*1477.32 +
    15301, written as int16, bitcast to fp16) for the rest. The
    Schraudolph multiplicative bias is mean-centered so softmax
    normalization cancels it; sawtooth ripple is ~1.8% rms on ~1/3 of
    tiles.
  - emission order interleaves the v-projection t-tiles and the NEXT
    pair's q/k projection units into the attention chunk stream, so
    the scalar engine starts exp ~25us earlier and PE has no idle
    window at pair boundaries. q/k units accumulate in [128,512] PSUM
    tiles (pool psB) so psA stays dedicated to S tiles.
  - output projection inlined per chunk inside pair-2's loop.

Layouts (unchanged): q^T/k^T [d, T] fp32r; S^T [tk, tq] via row-group
pairs; PV with [v | ones] stationaries so each head's softmax denominator
lands on the partitions opposite its O^T rows (normalize = reciprocal +
partition-swap DMA + multiply).
"""

import numpy as np

B, T, C = 4, 2048, 768
H = 12
D = C // H          # 64
HPC = 6             # heads per core
NP = 3              # head pairs per core
N_CORES = 8
TK = T // 128       # 16 tk tiles
NCH = T // 512      # 4 tq chunks
CT = C // 128       # 6 contraction tiles

SCH_A = 1477.319722   # 1024/ln2
SCH_B = 15301.086468  # 15*1024 - mean-centering constant

_cache = {}


def _build(has_bias):
    import concourse.tile as tile
    from concourse import bacc, mybir

    dt = mybir.dt
    r32 = dt.float32r
    f32 = dt.float32
    f16 = dt.float16
    i16 = dt.int16
    Exp = mybir.ActivationFunctionType.Exp
    Alu = mybir.AluOpType

    nc = bacc.Bacc("TRN2", target_bir_lowering=False, debug=False,
                   num_devices=N_CORES)

    xT_ap = nc.dram_tensor("xT", [C, T], f16, kind="ExternalInput").ap()
    wa_ap = nc.dram_tensor("wa", [C, 1152], f16, kind="ExternalInput").ap()
    wp_ap = nc.dram_tensor("wp", [HPC * D, C], r32, kind="ExternalInput").ap()
    nid_ap = nc.dram_tensor("nid", [128, 128], f16, kind="ExternalInput").ap()
    mkl_ap = nc.dram_tensor("mkl", [128, 128], f16, kind="ExternalInput").ap()
    if has_bias:
        ox_ap = nc.dram_tensor("ox", [1, T], f16, kind="ExternalInput").ap()
        wb_ap = nc.dram_tensor("wb", [1, 1152], f16, kind="ExternalInput").ap()
    out_ap = nc.dram_tensor("out", [T, C], f32, kind="ExternalOutput").ap()

    with tile.TileContext(nc) as tc:
        with tc.tile_pool(name="pers", bufs=1) as pers, \
             tc.tile_pool(name="pP", bufs=6) as pP, \
             tc.tile_pool(name="pst", bufs=2) as pst, \
             tc.tile_pool(name="pout", bufs=2) as pout, \
             tc.tile_pool(name="psA", bufs=2, space="PSUM") as psA, \
             tc.tile_pool(name="psB", bufs=4, space="PSUM") as psB:

            # ---- persistent SBUF tensors + input DMA ----
            xT = [pers.tile([128, T], f16, tag=f"x{i}", name=f"x{i}") for i in range(CT)]
            wa = [pers.tile([128, 1152], f16, tag=f"w{i}", name=f"w{i}") for i in range(CT)]
            for i in range(CT):
                nc.sync.dma_start(xT[i][:], xT_ap[i * 128:(i + 1) * 128, :])
                nc.sync.dma_start(wa[i][:], wa_ap[i * 128:(i + 1) * 128, :])
            wp = [pers.tile([128, C], r32, tag=f"p{i}", name=f"wp{i}") for i in range(NP)]
            for i in range(NP):
                nc.sync.dma_start(wp[i][:], wp_ap[i * 128:(i + 1) * 128, :])
            nid = pers.tile([128, 128], f16, tag="nid")
            nc.sync.dma_start(nid[:], nid_ap)
            mkl = pers.tile([128, 128], f16, tag="mkl")
            nc.sync.dma_start(mkl[:], mkl_ap)
            if has_bias:
                ox = pers.tile([1, T], f16, tag="ox")
                nc.sync.dma_start(ox[:], ox_ap)
                wb = pers.tile([1, 1152], f16, tag="wb")
                nc.sync.dma_start(wb[:], wb_ap)
            ones_f = pers.tile([128, 64], f32, tag="onesf")
            nc.vector.memset(ones_f[:], 1.0)
            ones_h = pers.tile([128, 64], f16, tag="onesh")
            nc.vector.tensor_copy(ones_h[:], ones_f[:])

            qk = [pers.tile([128, T], r32, tag=f"qk{i}", name=f"qk{i}") for i in range(2 * NP)]
            # v: one contiguous [128, 64] f16 tile per (t-tile, head) — 16-bit
            # stationaries must be dense tiles (sliced wide tiles hit a slow
            # non-FWL weight-load path).
            vv = [[pers.tile([128, 64], f16, tag=f"v{t}_{h}", name=f"v{t}_{h}")
                   for h in range(HPC)] for t in range(TK)]
            # Y^T for pair p: fresh tile for p=0; pairs 1 and 2 reuse the
            # q^T tiles of pairs 0 and 1, which are dead by the time attn
            # p starts writing (Tile tracks the WAR dependency).
            y0 = pers.tile([128, T], r32, tag="y0", name="y0")
            yt = [y0, qk[0], qk[2]]

            # ---- qkv projection emit helpers ----
            # q/k unit: one [128,512] PSUM accumulation (6 c-tile matmuls)
            # + one copy out.  8 units per pair; `on_act` routes the copy
            # to the scalar engine (used for pair 0, emitted in the idle
            # DMA-head window).
            def emit_qk_unit(p, u, on_act=False):
                qsel, cp2 = u // 4, u % 4      # qsel: 0=q 1=k; cp2: 512-col blk
                dst = qk[2 * p + qsel]
                wcol = qsel * 384 + p * 128
                t0 = cp2 * 512
                ps = psB.tile([128, 512], f32, tag="B")
                for c in range(CT):
                    nc.tensor.matmul(
                        ps[:], lhsT=wa[c][:, wcol:wcol + 128],
                        rhs=xT[c][:, t0:t0 + 512],
                        start=(c == 0),
                        stop=(c == CT - 1 and not has_bias))
                if has_bias:
                    nc.tensor.matmul(
                        ps[:], lhsT=wb[0:1, wcol:wcol + 128],
                        rhs=ox[0:1, t0:t0 + 512],
                        start=False, stop=True)
                if on_act:
                    nc.scalar.copy(dst[:, t0:t0 + 512], ps[:])
                else:
                    nc.vector.tensor_copy(dst[:, t0:t0 + 512], ps[:])

            # v unit: one t-tile -> six dense [128,64] head tiles.
            def emit_v_unit(t):
                ps = psB.tile([128, 512], f32, tag="B")
                for c in range(CT):
                    nc.tensor.matmul(
                        ps[:, 0:384],
                        lhsT=xT[c][:, t * 128:(t + 1) * 128],
                        rhs=wa[c][:, 768:1152],
                        start=(c == 0),
                        stop=(c == CT - 1 and not has_bias))
                if has_bias:
                    nc.tensor.matmul(
                        ps[:, 0:384],
                        lhsT=ox[0:1, t * 128:(t + 1) * 128],
                        rhs=wb[0:1, 768:1152],
                        start=False, stop=True)
                for h in range(HPC):
                    nc.vector.tensor_copy(
                        vv[t][h][:], ps[:, h * 64:(h + 1) * 64])

            def emit_proj(j):
                # proj for the 4 t-tiles covered by q-chunk j; needs yt of
                # all three pairs at those columns, which pair-2 chunk j's
                # normalize has just written.
                for t in range(4 * j, 4 * j + 4):
                    ob = pout.tile([128, C], f32, tag="o")
                    for (n0, n1) in ((0, 512), (512, 768)):
                        pp = psB.tile([128, 512], f32, tag="B")
                        for kk in range(NP):
                            nc.tensor.matmul(
                                pp[:, 0:n1 - n0],
                                lhsT=yt[kk][:, t * 128:(t + 1) * 128],
                                rhs=wp[kk][:, n0:n1],
                                start=(kk == 0), stop=(kk == NP - 1))
                        nc.vector.tensor_copy(ob[:, n0:n1], pp[:, 0:n1 - n0])
                    nc.sync.dma_start(
                        out_ap[t * 128:(t + 1) * 128, :], ob[:])

            # ---- phase 1 head: pair-0 q/k + first v tiles ----
            with nc.named_scope("qkv_qk"):
                for u in range(8):
                    emit_qk_unit(0, u, on_act=True)
            with nc.named_scope("qkv_v"):
                for t in range(4):
                    emit_v_unit(t)

            # ---- phase 2: attention per head pair ----
            sch_ctr = [0]

            for p in range(NP):
                qA = qk[2 * p]
                kA = qk[2 * p + 1]
                with nc.named_scope(f"attn{p}"):
                    for j in range(NCH):
                        nblk = 4 * j + 4
                        pvA = psB.tile([128, 512], f32, tag="B")
                        pvB = psB.tile([128, 512], f32, tag="B")
                        Ps = [None] * nblk
                        ms = [None] * nblk

                        def emit_S(i):
                            m = i - 4 * j
                            lo = 128 * m if m >= 0 else 0
                            sp = psA.tile([128, 1024], f32, tag="A")
                            for ab in range(2):
                                nc.tensor.matmul(
                                    sp[:, ab * 512 + lo:(ab + 1) * 512],
                                    lhsT=kA[ab * 64:(ab + 1) * 64,
                                            i * 128:(i + 1) * 128],
                                    rhs=qA[ab * 64:(ab + 1) * 64,
                                           j * 512 + lo:(j + 1) * 512],
                                    start=True, stop=(m < 0))
                            if m >= 0:
                                # fold the causal mask into the accumulation:
                                # adds -30 above the diagonal of the 128-wide
                                # diagonal sub-block; exp then gives 0.
                                for ab in range(2):
                                    nc.tensor.matmul(
                                        sp[:, ab * 512 + lo:ab * 512 + lo + 128],
                                        lhsT=nid[:], rhs=mkl[:],
                                        start=False, stop=True)
                            P = pP.tile([128, 1024], f16, tag="P")
                            if m < 0 and sch_ctr[0] % 3 == 2:
                                # Schraudolph exp on the Vector engine:
                                # fp16 bits = S*1024/ln2 + B, via int16.
                                nc.vector.tensor_scalar(
                                    out=P[:].bitcast(i16), in0=sp[:],
                                    scalar1=SCH_A, scalar2=SCH_B,
                                    op0=Alu.mult, op1=Alu.add)
                            elif lo == 0:
                                nc.scalar.activation(P[:], sp[:], Exp)
                            elif lo <= 256:
                                nc.scalar.activation(
                                    P[:, lo:1024], sp[:, lo:1024], Exp)
                            else:
                                nc.scalar.activation(
                                    P[:, lo:512], sp[:, lo:512], Exp)
                                nc.scalar.activation(
                                    P[:, 512 + lo:1024], sp[:, 512 + lo:1024],
                                    Exp)
                            if m < 0:
                                sch_ctr[0] += 1
                            Ps[i], ms[i] = P, max(m, 0)

                        def emit_PV(i):
                            m = ms[i]
                            lo = 128 * m
                            P = Ps[i]
                            first, last = (i == 0), (i == nblk - 1)
                            # head A: O on partitions 0-63, denom (ones) on
                            # 64-127; head B mirrored. O and denom matmuls
                            # land in different column groups and overlap.
                            nc.tensor.matmul(
                                pvA[0:64, lo:512], lhsT=vv[i][2 * p][:],
                                rhs=P[:, lo:512], start=first, stop=last)
                            nc.tensor.matmul(
                                pvA[64:128, lo:512], lhsT=ones_h[:],
                                rhs=P[:, lo:512], start=first, stop=last)
                            nc.tensor.matmul(
                                pvB[0:64, lo:512], lhsT=ones_h[:],
                                rhs=P[:, 512 + lo:1024], start=first, stop=last)
                            nc.tensor.matmul(
                                pvB[64:128, lo:512], lhsT=vv[i][2 * p + 1][:],
                                rhs=P[:, 512 + lo:1024], start=first, stop=last)

                        # software-pipeline: S(i+1) is emitted before PV(i)
                        emit_S(0)
                        for i in range(1, nblk):
                            emit_S(i)
                            emit_PV(i - 1)
                        emit_PV(nblk - 1)

                        # normalize. reciprocal_approx_fast only works at
                        # base partition 0, so: head B's denom (rows 0-63)
                        # is recip'd in place; head A's denom (rows 64-127)
                        # is staged, swapped down via DMA, then recip'd.
                        s1 = pst.tile([128, 512], f32, tag="st")
                        nc.vector.tensor_copy(s1[64:128, :], pvA[64:128, :])
                        nc.vector.reciprocal_approx_fast(
                            s1[0:64, :], pvB[0:64, :])
                        s2 = pst.tile([128, 512], f32, tag="rc")
                        nc.sync.dma_start(s2[0:64, :], s1[64:128, :])
                        nc.sync.dma_start(s2[64:128, :], s1[0:64, :])
                        s3 = pst.tile([64, 512], f32, tag="s3")
                        nc.vector.reciprocal_approx_fast(
                            s3[:], s2[0:64, :])
                        nc.vector.tensor_mul(
                            yt[p][0:64, j * 512:(j + 1) * 512],
                            pvA[0:64, :], s3[:])
                        nc.vector.tensor_mul(
                            yt[p][64:128, j * 512:(j + 1) * 512],
                            pvB[64:128, :], s2[64:128, :])

                        # interleave the rest of qkv + proj into the
                        # attention stream to keep PE fed and the scalar
                        # engine free of idle windows at pair boundaries.
                        if p == 0:
                            if j < 3:
                                with nc.named_scope("qkv_v"):
                                    for t in range(4 * j + 4, 4 * j + 8):
                                        emit_v_unit(t)
                            with nc.named_scope("qkv_qk"):
                                emit_qk_unit(1, 2 * j, on_act=False)
                                emit_qk_unit(1, 2 * j + 1, on_act=False)
                        elif p == 1:
                            with nc.named_scope("qkv_qk"):
                                emit_qk_unit(2, 2 * j, on_act=False)
                                emit_qk_unit(2, 2 * j + 1, on_act=False)
                        else:
                            with nc.named_scope("proj"):
                                emit_proj(j)

    nc.compile()
    return nc


def _prep_inputs(x, W_qkv, b_qkv, W_proj):
    """Per-core input maps (numpy; xT/wa as float16)."""
    sc = 1.0 / np.sqrt(D)
    nid = (-30.0 * np.eye(128)).astype(np.float16)
    mkl = np.tril(np.ones((128, 128)), -1).astype(np.float16)
    in_maps = []
    for c in range(N_CORES):
        b, hh = c // 2, c % 2
        h0 = hh * 384                      # column offset of this half's heads
        wq = W_qkv[:, h0:h0 + 384] * sc
        wk = W_qkv[:, 768 + h0:768 + h0 + 384]
        wv = W_qkv[:, 1536 + h0:1536 + h0 + 384]
        wa = np.ascontiguousarray(
            np.concatenate([wq, wk, wv], axis=1), dtype=np.float16)
        m = {
            "xT": np.ascontiguousarray(x[b].T, dtype=np.float16),
            "wa": wa,
            "wp": np.ascontiguousarray(W_proj[h0:h0 + 384, :], np.float32),
            "nid": nid,
            "mkl": mkl,
        }
        if np.any(b_qkv):
            bq = b_qkv[h0:h0 + 384] * sc
            bk = b_qkv[768 + h0:768 + h0 + 384]
            bv = b_qkv[1536 + h0:1536 + h0 + 384]
            m["ox"] = np.ones((1, T), dtype=np.float16)
            m["wb"] = np.concatenate([bq, bk, bv]).reshape(1, 1152).astype(
                np.float16)
        in_maps.append(m)
    return in_maps


def _run(inputs, trace=False, tmpdir=None):
    from concourse.bass_utils import run_bass_kernel_spmd

    x = np.asarray(inputs["x"], dtype=np.float32)
    W_qkv = np.asarray(inputs["W_qkv"], dtype=np.float32)
    b_qkv = np.asarray(inputs["b_qkv"], dtype=np.float32)
    W_proj = np.asarray(inputs["W_proj"], dtype=np.float32)
    b_proj = np.asarray(inputs["b_proj"], dtype=np.float32)

    has_bias = bool(np.any(b_qkv))
    key = ("k", has_bias)
    if key not in _cache:
        _cache[key] = _build(has_bias)
    nc = _cache[key]

    in_maps = _prep_inputs(x, W_qkv, b_qkv, W_proj)
    res = run_bass_kernel_spmd(nc, in_maps, list(range(N_CORES)),
                               trace=trace, tmpdir=tmpdir)
    out = np.empty((B, T, C), dtype=np.float32)
    for b in range(B):
        out[b] = res.results[2 * b]["out"] + res.results[2 * b + 1]["out"]
    out += b_proj
    return out, res


def kernel(**inputs):
    out, _ = _run(inputs)
    return out


# revision 4
# speedup vs baseline: 1.1988x; 1.1258x over previous
"""Causal self-attention (B=4, T=2048, C=768, H=12) on 8 trn2 NeuronCores.

Sharding: core c -> batch b = c//2, head-half hh = c%2 (6 heads per core).
Each core computes, for its (b, 6 heads): qkv projection, causal attention,
and a partial output projection (its heads' rows of W_proj). The host sums
the two partial outputs per batch and adds b_proj.

v3 over the 321us baseline:
  - all matmul operands in float16 (full PE rate, FWL-eligible 256B
    weight loads instead of 512B fp32r loads that were ~2x slower and
    exposed on the issue stream; quantization ~5e-4 per tensor).
  - PV uses merged [v | ones] (even heads) / [ones | v] (odd heads)
    [128,128] stationaries: ONE M=128 matmul per head per block computes
    both O^T and the softmax denominator — half the PV matmul and
    weight-load count of the split-M=64 scheme.
  - causal masking of diagonal 128-blocks happens INSIDE the S matmul
    accumulation group: an extra N=128 matmul adds -30 to the
    above-diagonal entries (lhsT = -30*I, rhs = strict-lower pattern),
    so exp() maps them to ~0 and no post-exp mask op exists.
  - softmax exp split across engines: ACT exp for diagonal blocks +
    2/3 of clean blocks; Schraudolph bits-of-fp16 exp (one Vector
    tensor_scalar: bits = S*1477.32 + 15301, int16 out, bitcast fp16)
    for the remaining third. Mean-centered so softmax cancels the bias.
  - emission interleaves v-projection t-tiles and the next pair's q/k
    units into the attention chunk stream; q/k PSUM->SBUF copies run
    on the scalar engine (its idle windows), v copies on Vector.
  - output projection inlined per chunk inside pair-2's loop.

Layout: q^T/k^T [d, T] fp16; S^T [tk, tq] blocks via row-group pairs
(head A on partitions 0-63, head B on 64-127); each head's softmax
denominator lands on the partitions opposite its O^T rows (normalize =
reciprocal + partition-swap DMA + multiply).
"""

import numpy as np

B, T, C = 4, 2048, 768
H = 12
D = C // H          # 64
HPC = 6             # heads per core
NP = 3              # head pairs per core
N_CORES = 8
TK = T // 128       # 16 tk tiles
NCH = T // 512      # 4 tq chunks
CT = C // 128       # 6 contraction tiles

SCH_A = 1477.319722   # 1024/ln2
SCH_B = 15301.086468  # 15*1024 - mean-centering constant

_cache = {}


def _build(has_bias):
    import concourse.tile as tile
    from concourse import bacc, mybir

    dt = mybir.dt
    f32 = dt.float32
    f16 = dt.float16
    i16 = dt.int16
    Exp = mybir.ActivationFunctionType.Exp
    Alu = mybir.AluOpType

    nc = bacc.Bacc("TRN2", target_bir_lowering=False, debug=False,
                   num_devices=N_CORES)

    xT_ap = nc.dram_tensor("xT", [C, T], f16, kind="ExternalInput").ap()
    wa_ap = nc.dram_tensor("wa", [C, 1152], f16, kind="ExternalInput").ap()
    wp_ap = nc.dram_tensor("wp", [HPC * D, C], f16, kind="ExternalInput").ap()
    nid_ap = nc.dram_tensor("nid", [128, 128], f16, kind="ExternalInput").ap()
    mkl_ap = nc.dram_tensor("mkl", [128, 128], f16, kind="ExternalInput").ap()
    if has_bias:
        ox_ap = nc.dram_tensor("ox", [1, T], f16, kind="ExternalInput").ap()
        wb_ap = nc.dram_tensor("wb", [1, 1152], f16, kind="ExternalInput").ap()
    out_ap = nc.dram_tensor("out", [T, C], f32, kind="ExternalOutput").ap()

    with tile.TileContext(nc) as tc:
        with tc.tile_pool(name="pers", bufs=1) as pers, \
             tc.tile_pool(name="pP", bufs=6) as pP, \
             tc.tile_pool(name="pst", bufs=2) as pst, \
             tc.tile_pool(name="pout", bufs=2) as pout, \
             tc.tile_pool(name="psA", bufs=2, space="PSUM") as psA, \
             tc.tile_pool(name="psB", bufs=4, space="PSUM") as psB:

            # ---- persistent SBUF tensors + input DMA ----
            xT = [pers.tile([128, T], f16, tag=f"x{i}", name=f"x{i}") for i in range(CT)]
            wa = [pers.tile([128, 1152], f16, tag=f"w{i}", name=f"w{i}") for i in range(CT)]
            for i in range(CT):
                nc.sync.dma_start(xT[i][:], xT_ap[i * 128:(i + 1) * 128, :])
                nc.sync.dma_start(wa[i][:], wa_ap[i * 128:(i + 1) * 128, :])
            wp = [pers.tile([128, C], f16, tag=f"p{i}", name=f"wp{i}") for i in range(NP)]
            for i in range(NP):
                nc.sync.dma_start(wp[i][:], wp_ap[i * 128:(i + 1) * 128, :])
            nid = pers.tile([128, 128], f16, tag="nid")
            nc.sync.dma_start(nid[:], nid_ap)
            mkl = pers.tile([128, 128], f16, tag="mkl")
            nc.sync.dma_start(mkl[:], mkl_ap)
            if has_bias:
                ox = pers.tile([1, T], f16, tag="ox")
                nc.sync.dma_start(ox[:], ox_ap)
                wb = pers.tile([1, 1152], f16, tag="wb")
                nc.sync.dma_start(wb[:], wb_ap)

            qk = [pers.tile([128, T], f16, tag=f"qk{i}", name=f"qk{i}") for i in range(2 * NP)]
            # PV stationaries: [128,128] per (t-tile, head): v in one column
            # half, ones in the other.  Even heads (pass A, out partitions
            # 0-63 = O^T, 64-127 = denom): [v | ones]; odd heads mirrored.
            # The ones halves are memset once on the (idle) gpsimd engine.
            vo = [[pers.tile([128, 128], f16, tag=f"v{t}_{h}", name=f"v{t}_{h}")
                   for h in range(HPC)] for t in range(TK)]
            for t in range(TK):
                for h in range(HPC):
                    oc = 64 if h % 2 == 0 else 0
                    nc.gpsimd.memset(vo[t][h][:, oc:oc + 64], 1.0)
            # Y^T for pair p: fresh tile for p=0; pairs 1 and 2 reuse the
            # q^T tiles of pairs 0 and 1, which are dead by the time attn
            # p starts writing (Tile tracks the WAR dependency).
            y0 = pers.tile([128, T], f16, tag="y0", name="y0")
            yt = [y0, qk[0], qk[2]]

            # ---- qkv projection emit helpers ----
            # q/k unit: one [128,512] PSUM accumulation (6 c-tile matmuls)
            # + one copy out on the scalar engine (whose idle windows line
            # up with when these units run).
            def emit_qk_unit(p, u):
                qsel, cp2 = u // 4, u % 4      # qsel: 0=q 1=k; cp2: 512-col blk
                dst = qk[2 * p + qsel]
                wcol = qsel * 384 + p * 128
                t0 = cp2 * 512
                ps = psB.tile([128, 512], f32, tag="B")
                for c in range(CT):
                    nc.tensor.matmul(
                        ps[:], lhsT=wa[c][:, wcol:wcol + 128],
                        rhs=xT[c][:, t0:t0 + 512],
                        start=(c == 0),
                        stop=(c == CT - 1 and not has_bias))
                if has_bias:
                    nc.tensor.matmul(
                        ps[:], lhsT=wb[0:1, wcol:wcol + 128],
                        rhs=ox[0:1, t0:t0 + 512],
                        start=False, stop=True)
                nc.scalar.copy(dst[:, t0:t0 + 512], ps[:])

            # v unit: one t-tile -> six [128,64] halves of the vo tiles.
            def emit_v_unit(t):
                ps = psB.tile([128, 512], f32, tag="B")
                for c in range(CT):
                    nc.tensor.matmul(
                        ps[:, 0:384],
                        lhsT=xT[c][:, t * 128:(t + 1) * 128],
                        rhs=wa[c][:, 768:1152],
                        start=(c == 0),
                        stop=(c == CT - 1 and not has_bias))
                if has_bias:
                    nc.tensor.matmul(
                        ps[:, 0:384],
                        lhsT=ox[0:1, t * 128:(t + 1) * 128],
                        rhs=wb[0:1, 768:1152],
                        start=False, stop=True)
                for h in range(HPC):
                    vc = 0 if h % 2 == 0 else 64
                    nc.vector.tensor_copy(
                        vo[t][h][:, vc:vc + 64], ps[:, h * 64:(h + 1) * 64])

            def emit_proj(j):
                # proj for the 4 t-tiles covered by q-chunk j; needs yt of
                # all three pairs at those columns, which pair-2 chunk j's
                # normalize has just written.
                for t in range(4 * j, 4 * j + 4):
                    ob = pout.tile([128, C], f32, tag="o")
                    for (n0, n1) in ((0, 512), (512, 768)):
                        pp = psB.tile([128, 512], f32, tag="B")
                        for kk in range(NP):
                            nc.tensor.matmul(
                                pp[:, 0:n1 - n0],
                                lhsT=yt[kk][:, t * 128:(t + 1) * 128],
                                rhs=wp[kk][:, n0:n1],
                                start=(kk == 0), stop=(kk == NP - 1))
                        nc.vector.tensor_copy(ob[:, n0:n1], pp[:, 0:n1 - n0])
                    nc.sync.dma_start(
                        out_ap[t * 128:(t + 1) * 128, :], ob[:])

            # ---- phase 1 head: pair-0 q/k + first v tiles ----
            with nc.named_scope("qkv_qk"):
                for u in range(8):
                    emit_qk_unit(0, u)
            with nc.named_scope("qkv_v"):
                for t in range(4):
                    emit_v_unit(t)

            # ---- phase 2: attention per head pair ----
            sch_ctr = [0]

            for p in range(NP):
                qA = qk[2 * p]
                kA = qk[2 * p + 1]
                with nc.named_scope(f"attn{p}"):
                    for j in range(NCH):
                        nblk = 4 * j + 4
                        pvA = psB.tile([128, 512], f32, tag="B")
                        pvB = psB.tile([128, 512], f32, tag="B")
                        Ps = [None] * nblk
                        ms = [None] * nblk

                        def emit_S(i):
                            m = i - 4 * j
                            lo = 128 * m if m >= 0 else 0
                            sp = psA.tile([128, 1024], f32, tag="A")
                            for ab in range(2):
                                nc.tensor.matmul(
                                    sp[:, ab * 512 + lo:(ab + 1) * 512],
                                    lhsT=kA[ab * 64:(ab + 1) * 64,
                                            i * 128:(i + 1) * 128],
                                    rhs=qA[ab * 64:(ab + 1) * 64,
                                           j * 512 + lo:(j + 1) * 512],
                                    start=True, stop=(m < 0))
                            if m >= 0:
                                # fold the causal mask into the accumulation:
                                # adds -30 above the diagonal of the 128-wide
                                # diagonal sub-block; exp then gives ~0.
                                for ab in range(2):
                                    nc.tensor.matmul(
                                        sp[:, ab * 512 + lo:ab * 512 + lo + 128],
                                        lhsT=nid[:], rhs=mkl[:],
                                        start=False, stop=True)
                            P = pP.tile([128, 1024], f16, tag="P")
                            if m < 0 and sch_ctr[0] % 3 == 2:
                                # Schraudolph exp on the Vector engine:
                                # fp16 bits = S*1024/ln2 + B, via int16.
                                nc.vector.tensor_scalar(
                                    out=P[:].bitcast(i16), in0=sp[:],
                                    scalar1=SCH_A, scalar2=SCH_B,
                                    op0=Alu.mult, op1=Alu.add)
                            elif lo == 0:
                                nc.scalar.activation(P[:], sp[:], Exp)
                            elif lo <= 256:
                                nc.scalar.activation(
                                    P[:, lo:1024], sp[:, lo:1024], Exp)
                            else:
                                nc.scalar.activation(
                                    P[:, lo:512], sp[:, lo:512], Exp)
                                nc.scalar.activation(
                                    P[:, 512 + lo:1024], sp[:, 512 + lo:1024],
                                    Exp)
                            if m < 0:
                                sch_ctr[0] += 1
                            Ps[i], ms[i] = P, max(m, 0)

                        def emit_PV(i):
                            m = ms[i]
                            lo = 128 * m
                            P = Ps[i]
                            first, last = (i == 0), (i == nblk - 1)
                            # merged stationaries: one matmul per head gives
                            # O^T on one partition half and the denominator
                            # on the other.
                            nc.tensor.matmul(
                                pvA[:, lo:512], lhsT=vo[i][2 * p][:],
                                rhs=P[:, lo:512], start=first, stop=last)
                            nc.tensor.matmul(
                                pvB[:, lo:512], lhsT=vo[i][2 * p + 1][:],
                                rhs=P[:, 512 + lo:1024], start=first, stop=last)

                        # software-pipeline: S(i+1) is emitted before PV(i)
                        emit_S(0)
                        for i in range(1, nblk):
                            emit_S(i)
                            emit_PV(i - 1)
                        emit_PV(nblk - 1)

                        # normalize. reciprocal_approx_fast only works at
                        # base partition 0, so: head B's denom (rows 0-63)
                        # is recip'd in place; head A's denom (rows 64-127)
                        # is staged, swapped down via DMA, then recip'd.
                        s1 = pst.tile([128, 512], f32, tag="st")
                        nc.vector.tensor_copy(s1[64:128, :], pvA[64:128, :])
                        nc.vector.reciprocal_approx_fast(
                            s1[0:64, :], pvB[0:64, :])
                        s2 = pst.tile([128, 512], f32, tag="rc")
                        nc.sync.dma_start(s2[0:64, :], s1[64:128, :])
                        nc.sync.dma_start(s2[64:128, :], s1[0:64, :])
                        s3 = pst.tile([64, 512], f32, tag="s3")
                        nc.vector.reciprocal_approx_fast(
                            s3[:], s2[0:64, :])
                        nc.vector.tensor_mul(
                            yt[p][0:64, j * 512:(j + 1) * 512],
                            pvA[0:64, :], s3[:])
                        nc.vector.tensor_mul(
                            yt[p][64:128, j * 512:(j + 1) * 512],
                            pvB[64:128, :], s2[64:128, :])

                        # interleave the rest of qkv + proj into the
                        # attention stream to keep PE fed and the scalar
                        # engine free of idle windows at pair boundaries.
                        if p == 0:
                            if j < 3:
                                with nc.named_scope("qkv_v"):
                                    for t in range(4 * j + 4, 4 * j + 8):
                                        emit_v_unit(t)
                            with nc.named_scope("qkv_qk"):
                                emit_qk_unit(1, 2 * j)
                                emit_qk_unit(1, 2 * j + 1)
                        elif p == 1:
                            with nc.named_scope("qkv_qk"):
                                emit_qk_unit(2, 2 * j)
                                emit_qk_unit(2, 2 * j + 1)
                        else:
                            with nc.named_scope("proj"):
                                emit_proj(j)

    nc.compile()
    return nc


def _prep_inputs(x, W_qkv, b_qkv, W_proj):
    """Per-core input maps (numpy; all matmul operands float16)."""
    sc = 1.0 / np.sqrt(D)
    nid = (-30.0 * np.eye(128)).astype(np.float16)
    mkl = np.tril(np.ones((128, 128)), -1).astype(np.float16)
    in_maps = []
    for c in range(N_CORES):
        b, hh = c // 2, c % 2
        h0 = hh * 384                      # column offset of this half's heads
        wq = W_qkv[:, h0:h0 + 384] * sc
        wk = W_qkv[:, 768 + h0:768 + h0 + 384]
        wv = W_qkv[:, 1536 + h0:1536 + h0 + 384]
        wa = np.ascontiguousarray(
            np.concatenate([wq, wk, wv], axis=1), dtype=np.float16)
        m = {
            "xT": np.ascontiguousarray(x[b].T, dtype=np.float16),
            "wa": wa,
            "wp": np.ascontiguousarray(W_proj[h0:h0 + 384, :], np.float16),
            "nid": nid,
            "mkl": mkl,
        }
        if np.any(b_qkv):
            bq = b_qkv[h0:h0 + 384] * sc
            bk = b_qkv[768 + h0:768 + h0 + 384]
            bv = b_qkv[1536 + h0:1536 + h0 + 384]
            m["ox"] = np.ones((1, T), dtype=np.float16)
            m["wb"] = np.concatenate([bq, bk, bv]).reshape(1, 1152).astype(
                np.float16)
        in_maps.append(m)
    return in_maps


def _run(inputs, trace=False, tmpdir=None):
    from concourse.bass_utils import run_bass_kernel_spmd

    x = np.asarray(inputs["x"], dtype=np.float32)
    W_qkv = np.asarray(inputs["W_qkv"], dtype=np.float32)
    b_qkv = np.asarray(inputs["b_qkv"], dtype=np.float32)
    W_proj = np.asarray(inputs["W_proj"], dtype=np.float32)
    b_proj = np.asarray(inputs["b_proj"], dtype=np.float32)

    has_bias = bool(np.any(b_qkv))
    key = ("k", has_bias)
    if key not in _cache:
        _cache[key] = _build(has_bias)
    nc = _cache[key]

    in_maps = _prep_inputs(x, W_qkv, b_qkv, W_proj)
    res = run_bass_kernel_spmd(nc, in_maps, list(range(N_CORES)),
                               trace=trace, tmpdir=tmpdir)
    out = np.empty((B, T, C), dtype=np.float32)
    for b in range(B):
        out[b] = res.results[2 * b]["out"] + res.results[2 * b + 1]["out"]
    out += b_proj
    return out, res


def kernel(**inputs):
    out, _ = _run(inputs)
    return out
